# revision 4
# baseline (speedup 1.0000x reference)
"""MoE block (B=16,N=1024,C=768,E=8,H=192,D=4,K=2) on 8 NeuronCores.

Strategy: data-parallel over B (2 samples/core). Per sample, compute the
noisy gating on-device (split-bf16 3-matmul for fp32-grade accuracy), take
top-2 experts, gather only those experts' weights via indirect DMA, and run
the 2-layer MLP in bf16 (fp32 PSUM accumulate) with exact-Gelu, combining
with the top-2 gates and the fp32 residual.

Layouts shipped from host (pure value-preserving prep: shard, transpose,
bf16 split, index-gather of gate_w by task_ids):
  x_f32  [2,1024,768] f32   residual + exactness
  x_hi   [2,1024,768] bf16  = bf16(x)       (DMA-transposed on device)
  x_lo   [2,1024,768] bf16  = bf16(x - x_hi)
  gw_hi/gw_lo [2,768,16] bf16 split of gate_w[task_id]
  w1aug  [8*776,192] bf16: fc1_w[e].T rows + fc1_b row at 768 (+pad)
  w2aug  [8*193,768] bf16: fc2_w[e].T rows + fc2_b row at 192
  eps_t  [2,8,1024] f32
  id8    [8,8] f32
"""
import numpy as np
import ml_dtypes

import concourse.bass as bass
import concourse.mybir as mybir
import concourse.tile as tile
from concourse import bacc
from concourse.bass_utils import run_bass_kernel_spmd

bf16 = ml_dtypes.bfloat16
f32 = np.float32
AF = mybir.ActivationFunctionType
ALU = mybir.AluOpType
dt = mybir.dt

B, N, C = 16, 1024, 768
E, H, D, TOPK = 8, 192, 4, 2
NCORES = 8
SPC = B // NCORES          # samples per core = 2
C_K = C // 128             # 6 K-chunks over channels
W1_ROWS = C + 8            # 776: 768 wT rows + bias row + pad
W2_ROWS = H + 1            # 193
NT = N // 512              # 2 big n-chunks
TCH = N // 128             # 8 token chunks

_cache = {}


def _build():
    if "nc" in _cache:
        return _cache["nc"]
    nc = bacc.Bacc("TRN2", target_bir_lowering=False, debug=False,
                   num_devices=NCORES)

    xf_d = nc.dram_tensor("x_f32", [SPC, N, C], dt.float32, kind="ExternalInput").ap()
    xh_d = nc.dram_tensor("x_hi", [SPC, N, C], dt.bfloat16, kind="ExternalInput").ap()
    xl_d = nc.dram_tensor("x_lo", [SPC, N, C], dt.bfloat16, kind="ExternalInput").ap()
    gh_d = nc.dram_tensor("gw_hi", [SPC, C, 40], dt.bfloat16, kind="ExternalInput").ap()
    gl_d = nc.dram_tensor("gw_lo", [SPC, C, 40], dt.bfloat16, kind="ExternalInput").ap()
    w1_d = nc.dram_tensor("w1aug", [E * W1_ROWS, H], dt.bfloat16, kind="ExternalInput").ap()
    w2_d = nc.dram_tensor("w2aug", [E * W2_ROWS, C], dt.bfloat16, kind="ExternalInput").ap()
    ep_d = nc.dram_tensor("eps_t", [SPC, E, N], dt.float32, kind="ExternalInput").ap()
    id_d = nc.dram_tensor("id8", [E, E], dt.float32, kind="ExternalInput").ap()
    y_d = nc.dram_tensor("y", [SPC, N, C], dt.float32, kind="ExternalOutput").ap()

    with tile.TileContext(nc) as tc:
        with tc.tile_pool(name="const", bufs=1) as cp, \
             tc.tile_pool(name="xt", bufs=2) as xtp, \
             tc.tile_pool(name="gw", bufs=2) as gwp, \
             tc.tile_pool(name="gate", bufs=2) as gp, \
             tc.tile_pool(name="w1", bufs=2) as w1p, \
             tc.tile_pool(name="w2", bufs=2) as w2p, \
             tc.tile_pool(name="h", bufs=2) as hp, \
             tc.tile_pool(name="xres", bufs=3) as xrp, \
             tc.tile_pool(name="yout", bufs=3) as yp, \
             tc.tile_pool(name="ps_g", bufs=2, space="PSUM") as psg, \
             tc.tile_pool(name="ps_f1", bufs=2, space="PSUM") as psf, \
             tc.tile_pool(name="ps_y", bufs=2, space="PSUM") as psy, \
             tc.tile_pool(name="ps_t", bufs=2, space="PSUM") as pst:

            # constants
            iota_f = cp.tile([128, 1], dt.float32, tag="iota_f")
            iota_i = cp.tile([128, 1], dt.int32, tag="iota_i")
            nc.gpsimd.iota(iota_i[:], pattern=[[0, 1]], base=0, channel_multiplier=1)
            nc.vector.tensor_copy(iota_f[:], iota_i[:])
            ones1 = cp.tile([1, 128], dt.float32, tag="ones1")
            nc.vector.memset(ones1[:], 1.0)
            id8 = cp.tile([E, E], dt.float32, tag="id8")
            nc.sync.dma_start(id8[:], id_d[:, :])
            xta = cp.tile([8, N], dt.bfloat16, tag="xta")   # aug ones chunk for fc1
            nc.vector.memset(xta[:], 0.0)
            nc.vector.memset(xta[0:1, :], 1.0)

            for s in range(SPC):
                # ---- A. transpose-load x (bf16 hi/lo) ----
                xT_hi = [xtp.tile([128, N], dt.bfloat16, tag=f"xh{k}", name=f"xh{k}") for k in range(C_K)]
                xT_lo = [xtp.tile([128, N], dt.bfloat16, tag=f"xl{k}", name=f"xl{k}") for k in range(C_K)]
                for k in range(C_K):
                    nc.sync.dma_start_transpose(xT_hi[k][:], xh_d[s, :, 128 * k:128 * (k + 1)])
                    nc.sync.dma_start_transpose(xT_lo[k][:], xl_d[s, :, 128 * k:128 * (k + 1)])

                # ---- B. gating matmuls: [16, N] = gwT @ x ----
                gwh = [gwp.tile([128, 40], dt.bfloat16, tag=f"gwh{k}", name=f"gwh{k}") for k in range(C_K)]
                gwl = [gwp.tile([128, 40], dt.bfloat16, tag=f"gwl{k}", name=f"gwl{k}") for k in range(C_K)]
                for k in range(C_K):
                    nc.sync.dma_start(gwh[k][:], gh_d[s, 128 * k:128 * (k + 1), :])
                    nc.sync.dma_start(gwl[k][:], gl_d[s, 128 * k:128 * (k + 1), :])
                gt = []
                for n in range(NT):
                    g_ps = psg.tile([40, 512], dt.float32, space="PSUM", tag="gps")
                    first = True
                    for (lw, rx) in ((gwh, xT_hi), (gwh, xT_lo), (gwl, xT_hi)):
                        for k in range(C_K):
                            nc.tensor.matmul(
                                out=g_ps[:], lhsT=lw[k][:],
                                rhs=rx[k][:, 512 * n:512 * (n + 1)],
                                start=first, stop=(lw is gwl and k == C_K - 1))
                            first = False
                    gt.append(g_ps)

                # ---- C. ews = sum_n clean + sum_n eps*(softplus(noise)+0.01) ----
                epsT = gp.tile([E, N], dt.float32, tag="epsT")
                nc.sync.dma_start(epsT[:], ep_d[s, :, :])
                reds = []
                for n in range(NT):
                    ex = gp.tile([E, 512], dt.float32, tag="ex")
                    nc.scalar.activation(ex[:], gt[n][32:40, :], AF.Exp)
                    sp = gp.tile([E, 512], dt.float32, tag="sp")
                    nc.scalar.activation(sp[:], ex[:], AF.Ln, bias=1.0)
                    stdp = gp.tile([E, 512], dt.float32, tag="stdp")
                    nc.vector.tensor_scalar_add(stdp[:], sp[:], 0.01)
                    prod = gp.tile([E, 512], dt.float32, tag="prod")
                    nc.vector.tensor_tensor(out=prod[:], in0=stdp[:],
                                            in1=epsT[:, 512 * n:512 * (n + 1)], op=ALU.mult)
                    rn = gp.tile([E, 1], dt.float32, tag=f"rn{n}")
                    nc.vector.tensor_reduce(out=rn[:], in_=prod[:],
                                            axis=mybir.AxisListType.X, op=ALU.add)
                    rc = gp.tile([E, 1], dt.float32, tag=f"rc{n}")
                    nc.vector.tensor_reduce(out=rc[:], in_=gt[n][0:E, :],
                                            axis=mybir.AxisListType.X, op=ALU.add)
                    reds.append((rn, rc))
                ews = gp.tile([E, 1], dt.float32, tag="ews")
                nc.vector.tensor_add(ews[:], reds[0][0][:], reds[0][1][:])
                nc.vector.tensor_add(ews[:], ews[:], reds[1][0][:])
                nc.vector.tensor_add(ews[:], ews[:], reds[1][1][:])

                # ---- D. top-2 + gates, broadcast to 128 partitions ----
                r_ps = pst.tile([1, E], dt.float32, space="PSUM", tag="tps")
                nc.tensor.matmul(out=r_ps[:], lhsT=ews[:], rhs=id8[:], start=True, stop=True)
                ews_row = gp.tile([1, E], dt.float32, tag="ews_row")
                nc.vector.tensor_copy(ews_row[:], r_ps[:])
                b_ps = pst.tile([128, E], dt.float32, space="PSUM", tag="tps")
                nc.tensor.matmul(out=b_ps[:], lhsT=ones1[:], rhs=ews_row[:], start=True, stop=True)
                ewsb = gp.tile([128, E], dt.float32, tag="ewsb")
                nc.vector.tensor_copy(ewsb[:], b_ps[:])
                mx = gp.tile([128, 8], dt.float32, tag="mx")
                mi = gp.tile([128, 8], dt.uint32, tag="mi")
                nc.vector.max_with_indices(mx[:], mi[:], ewsb[:])
                dd = gp.tile([128, 1], dt.float32, tag="dd")
                nc.vector.tensor_sub(dd[:], mx[:, 0:1], mx[:, 1:2])
                den = gp.tile([128, 1], dt.float32, tag="den")
                nc.vector.tensor_scalar_add(den[:], dd[:], 1e-6)
                rec = gp.tile([128, 1], dt.float32, tag="rec")
                nc.vector.reciprocal(rec[:], den[:])
                s1 = gp.tile([128, 1], dt.float32, tag="s1")
                nc.vector.tensor_tensor(out=s1[:], in0=dd[:], in1=rec[:], op=ALU.mult)
                et = gp.tile([128, 1], dt.float32, tag="et")
                nc.scalar.activation(et[:], s1[:], AF.Exp, scale=-1.0)
                den2 = gp.tile([128, 1], dt.float32, tag="den2")
                nc.vector.tensor_scalar_add(den2[:], et[:], 1.0)
                g1 = gp.tile([128, 1], dt.float32, tag="g1")
                nc.vector.reciprocal(g1[:], den2[:])
                g2 = gp.tile([128, 1], dt.float32, tag="g2")
                nc.vector.tensor_tensor(out=g2[:], in0=et[:], in1=g1[:], op=ALU.mult)

                # ---- E. experts: gather weights + fc1 + gelu + scale ----
                hTs = []
                for j in range(TOPK):
                    g_col = g1 if j == 0 else g2
                    idxf = gp.tile([128, 1], dt.float32, tag=f"idxf{j}")
                    nc.vector.tensor_copy(idxf[:], mi[:, j:j + 1])
                    b1f = gp.tile([128, 1], dt.float32, tag=f"b1f{j}")
                    nc.vector.tensor_scalar(out=b1f[:], in0=idxf[:], scalar1=float(W1_ROWS),
                                            scalar2=None, op0=ALU.mult)
                    nc.vector.tensor_add(b1f[:], b1f[:], iota_f[:])
                    b2f = gp.tile([128, 1], dt.float32, tag=f"b2f{j}")
                    nc.vector.tensor_scalar(out=b2f[:], in0=idxf[:], scalar1=float(W2_ROWS),
                                            scalar2=None, op0=ALU.mult)
                    nc.vector.tensor_add(b2f[:], b2f[:], iota_f[:])

                    w1t = []
                    for k in range(C_K):
                        gi = gp.tile([128, 1], dt.uint32, tag=f"gi1_{j}_{k}")
                        if k == 0:
                            nc.vector.tensor_copy(gi[:], b1f[:])
                        else:
                            gf = gp.tile([128, 1], dt.float32, tag=f"gf1_{j}_{k}")
                            nc.vector.tensor_scalar_add(gf[:], b1f[:], float(128 * k))
                            nc.vector.tensor_copy(gi[:], gf[:])
                        wt = w1p.tile([128, H], dt.bfloat16, tag=f"w1_{j}_{k}")
                        nc.gpsimd.indirect_dma_start(
                            out=wt[:], out_offset=None, in_=w1_d[:],
                            in_offset=bass.IndirectOffsetOnAxis(ap=gi[:, :1], axis=0))
                        w1t.append(wt)
                    gfb = gp.tile([8, 1], dt.float32, tag=f"gfb{j}")
                    nc.vector.tensor_scalar_add(gfb[:], b1f[0:8, :], float(C))
                    gib = gp.tile([8, 1], dt.uint32, tag=f"gib{j}")
                    nc.vector.tensor_copy(gib[:], gfb[:])
                    w1b = w1p.tile([8, H], dt.bfloat16, tag=f"w1b{j}")
                    nc.gpsimd.indirect_dma_start(
                        out=w1b[:], out_offset=None, in_=w1_d[:],
                        in_offset=bass.IndirectOffsetOnAxis(ap=gib[:, :1], axis=0))
                    w1t.append(w1b)

                    gi2 = gp.tile([128, 1], dt.uint32, tag=f"gi2{j}")
                    nc.vector.tensor_copy(gi2[:], b2f[:])
                    w2t = w2p.tile([128, C], dt.bfloat16, tag=f"w2_{j}")
                    nc.gpsimd.indirect_dma_start(
                        out=w2t[:], out_offset=None, in_=w2_d[:],
                        in_offset=bass.IndirectOffsetOnAxis(ap=gi2[:, :1], axis=0))
                    gf2b = gp.tile([H - 128 + 1, 1], dt.float32, tag=f"gf2b{j}")
                    nc.vector.tensor_scalar_add(gf2b[:], b2f[0:H - 128 + 1, :], 128.0)
                    gi2b = gp.tile([H - 128 + 1, 1], dt.uint32, tag=f"gi2b{j}")
                    nc.vector.tensor_copy(gi2b[:], gf2b[:])
                    w2b = w2p.tile([H - 128 + 1, C], dt.bfloat16, tag=f"w2b{j}")
                    nc.gpsimd.indirect_dma_start(
                        out=w2b[:], out_offset=None, in_=w2_d[:],
                        in_offset=bass.IndirectOffsetOnAxis(ap=gi2b[:, :1], axis=0))

                    # fc1 -> gelu -> scale by gate -> hT bf16 (aug row = gate)
                    hT0 = hp.tile([128, N], dt.bfloat16, tag=f"hT0_{j}")
                    hT1 = hp.tile([H - 128 + 1, N], dt.bfloat16, tag=f"hT1_{j}")
                    for n in range(NT):
                        for m in range(2):
                            msz = 128 if m == 0 else H - 128
                            f_ps = psf.tile([msz, 512], dt.float32, space="PSUM",
                                            tag="fps")
                            for k in range(C_K + 1):
                                rx = (xT_hi[k] if k < C_K else xta)
                                nc.tensor.matmul(
                                    out=f_ps[:],
                                    lhsT=w1t[k][:, 128 * m:128 * m + msz],
                                    rhs=rx[:, 512 * n:512 * (n + 1)],
                                    start=(k == 0), stop=(k == C_K))
                            gel = hp.tile([msz, 512], dt.float32, tag=f"gel{m}")
                            nc.scalar.activation(gel[:], f_ps[:], AF.Gelu)
                            dst = hT0 if m == 0 else hT1
                            nc.vector.tensor_scalar(
                                out=dst[0:msz, 512 * n:512 * (n + 1)], in0=gel[:],
                                scalar1=g_col[0:msz, :], scalar2=None, op0=ALU.mult)
                    nc.vector.tensor_copy(hT1[H - 128:H - 128 + 1, :],
                                          g_col[0:1, 0:1].to_broadcast([1, N]))
                    hTs.append((hT0, hT1, w2t, w2b))

                # ---- F. fc2 + residual + store, per 128-token chunk ----
                for t in range(TCH):
                    xr = xrp.tile([128, C], dt.float32, tag="xr")
                    nc.sync.dma_start(xr[:], xf_d[s, 128 * t:128 * (t + 1), :])
                    ys = yp.tile([128, C], dt.float32, tag="ys")
                    for c2 in range(2):
                        y_ps = psy.tile([128, 384], dt.float32, space="PSUM", tag="yps")
                        for j in range(TOPK):
                            hT0, hT1, w2t, w2b = hTs[j]
                            nc.tensor.matmul(
                                out=y_ps[:], lhsT=hT0[:, 128 * t:128 * (t + 1)],
                                rhs=w2t[:, 384 * c2:384 * (c2 + 1)],
                                start=(j == 0), stop=False)
                            nc.tensor.matmul(
                                out=y_ps[:], lhsT=hT1[:, 128 * t:128 * (t + 1)],
                                rhs=w2b[:, 384 * c2:384 * (c2 + 1)],
                                start=False, stop=(j == TOPK - 1))
                        nc.vector.tensor_add(ys[:, 384 * c2:384 * (c2 + 1)],
                                             xr[:, 384 * c2:384 * (c2 + 1)], y_ps[:])
                    nc.sync.dma_start(y_d[s, 128 * t:128 * (t + 1), :], ys[:])

    nc.compile()
    _cache["nc"] = nc
    return nc


def _prep_inputs(x, task_ids, eps, gate_w, fc1_w, fc1_b, fc2_w, fc2_b):
    x = np.ascontiguousarray(np.asarray(x, dtype=f32))
    task_ids = np.asarray(task_ids).astype(np.int64)
    eps = np.asarray(eps, dtype=f32)
    gate_w = np.asarray(gate_w, dtype=f32)
    x_hi = x.astype(bf16)
    x_lo = (x - x_hi.astype(f32)).astype(bf16)
    gw = gate_w[task_ids]                      # [B, C, 2E]
    gw40 = np.zeros((B, C, 40), dtype=f32)     # clean at cols 0:8, noise at 32:40
    gw40[..., 0:E] = gw[..., 0:E]
    gw40[..., 32:32 + E] = gw[..., E:2 * E]
    gw_hi = gw40.astype(bf16)
    gw_lo = (gw40 - gw_hi.astype(f32)).astype(bf16)
    eps_t = np.ascontiguousarray(np.swapaxes(eps, 1, 2))   # [B, E, N]

    w1aug = np.zeros((E, W1_ROWS, H), dtype=f32)
    w1aug[:, :C, :] = np.swapaxes(np.asarray(fc1_w, dtype=f32), 1, 2)
    w1aug[:, C, :] = np.asarray(fc1_b, dtype=f32)
    w1aug = w1aug.reshape(E * W1_ROWS, H).astype(bf16)
    w2aug = np.zeros((E, W2_ROWS, C), dtype=f32)
    w2aug[:, :H, :] = np.swapaxes(np.asarray(fc2_w, dtype=f32), 1, 2)
    w2aug[:, H, :] = np.asarray(fc2_b, dtype=f32)
    w2aug = w2aug.reshape(E * W2_ROWS, C).astype(bf16)
    id8 = np.eye(E, dtype=f32)

    in_maps = []
    for c in range(NCORES):
        sl = slice(SPC * c, SPC * (c + 1))
        in_maps.append({
            "x_f32": x[sl], "x_hi": x_hi[sl], "x_lo": x_lo[sl],
            "gw_hi": np.ascontiguousarray(gw_hi[sl]),
            "gw_lo": np.ascontiguousarray(gw_lo[sl]),
            "w1aug": w1aug, "w2aug": w2aug,
            "eps_t": eps_t[sl], "id8": id8,
        })
    return in_maps


def kernel(x, task_ids, eps, gate_w, fc1_w, fc1_b, fc2_w, fc2_b, _trace=False):
    nc = _build()
    in_maps = _prep_inputs(x, task_ids, eps, gate_w, fc1_w, fc1_b, fc2_w, fc2_b)
    res = run_bass_kernel_spmd(nc, in_maps, list(range(NCORES)), trace=_trace)
    out = np.concatenate([res.results[c]["y"] for c in range(NCORES)], axis=0)
    kernel.last_results = res
    return out.astype(np.float32)
